# revision 27
# baseline (speedup 1.0000x reference)
# kernel.py — fused causal ReLU-attention (qkv proj + q@k^T + relu/causal + @v)
# for Trainium2, 8 NeuronCores, batch-parallel (1 batch element per core).
#
# v2: host pre-transposes x (device gets x^T) and the device returns y^T,
# so the PE does zero transposes; exact causal trimming; one PSUM y tile
# per chunk (both head-halves as col-quadrants); pair-pipelined schedule
# that keeps K=128 qkv matmuls inside every attention window.
#
# Self-contained: hardcodes shapes B,T,C = 8,1024,768, nh=12, hs=64.
import os
import sys

for p in ("/opt/trn_rl_repo", "/root/.axon_site", "/root/.axon_site/_ro/trn_rl_repo"):
    if os.path.isdir(p) and p not in sys.path:
        sys.path.append(p)

import numpy as np

import concourse.bass as bass
import concourse.mybir as mybir
import concourse.tile as tile
from concourse import bacc
from concourse import bass_utils

F32 = mybir.dt.float32
BF16 = mybir.dt.bfloat16
AF = mybir.ActivationFunctionType
ALU = mybir.AluOpType

B, T, C = 8, 1024, 768
NH, HS = 12, 64
SCALE = 1.0 / 8.0  # 1/sqrt(64)
P = 128
NT = T // P      # 8 t-tiles
KC = C // P      # 6 c-tiles (contraction)
NPAIR = NH // 2  # 6 head pairs (one 128-wide m-tile per pair)
TCH = 512        # q-chunk width (one PSUM bank)
NCH = T // TCH   # 2 chunks


def build_nc(n_cores=8):
    nc = bacc.Bacc("TRN2", target_bir_lowering=False, debug=False,
                   num_devices=n_cores)

    # x arrives pre-transposed from host: xT = x^T  [C, T]
    xt_d = nc.dram_tensor("x", [C, T], BF16, kind="ExternalInput").ap()
    w_d = nc.dram_tensor("w", [C, 3 * C], BF16, kind="ExternalInput").ap()
    b_d = nc.dram_tensor("b", [3 * C], F32, kind="ExternalInput").ap()
    # y is produced transposed: yT = y^T  [C, T]; host transposes back
    y_d = nc.dram_tensor("y", [C, T], BF16, kind="ExternalOutput").ap()

    with tile.TileContext(nc) as tc:
        _emit(nc, tc, xt_d, w_d, b_d, y_d)

    nc.compile()
    return nc


def _emit(nc, tc, xt_d, w_d, b_d, y_d):
    from contextlib import ExitStack

    with ExitStack() as ctx:
        pp = ctx.enter_context(tc.tile_pool(name="persist", bufs=1))
        xtp = ctx.enter_context(tc.tile_pool(name="xT", bufs=1))
        wqk = ctx.enter_context(tc.tile_pool(name="wqk", bufs=1))
        wvp = ctx.enter_context(tc.tile_pool(name="wv", bufs=1))
        vp = ctx.enter_context(tc.tile_pool(name="vsb", bufs=1))
        at_pool = ctx.enter_context(tc.tile_pool(name="attp", bufs=24))
        yt_pool = ctx.enter_context(tc.tile_pool(name="yT", bufs=2))
        qps = ctx.enter_context(
            tc.tile_pool(name="qkv_psum", bufs=2, space="PSUM"))
        a_ps = ctx.enter_context(
            tc.tile_pool(name="att_psum", bufs=4, space="PSUM"))
        y_ps = ctx.enter_context(
            tc.tile_pool(name="y_psum", bufs=2, space="PSUM"))

        # ---- input DMAs, critical-first, spread over 3 HWDGE queues ----
        xT = [xtp.tile([P, T], BF16, tag=f"xT{k}", name=f"xT{k}")
              for k in range(KC)]
        w_sb = [wqk.tile([P, 2 * C], BF16, tag=f"w{k}", name=f"w{k}")
                for k in range(KC)]
        wv = [wvp.tile([P, C], BF16, tag=f"wv{k}", name=f"wv{k}")
              for k in range(KC)]
        bqk = pp.tile([P, 12], F32, tag="bqk", name="bqk")
        bv_row = pp.tile([1, C], F32, tag="bvrow", name="bvrow")

        def in_dma(eng, k):
            eng.dma_start(w_sb[k][:], w_d[P * k:P * (k + 1), 0:2 * C])
            eng.dma_start(xT[k][:], xt_d[P * k:P * (k + 1), :])

        # scratch memset must be the FIRST Pool-queue op: the HAM warm-up
        # matmul chain depends on it and must start during the preamble
        scratch = pp.tile([P, TCH], BF16, tag="scratch", name="scratch")
        nc.gpsimd.memset(scratch[:], 0.0)

        in_dma(nc.sync, 0)
        in_dma(nc.scalar, 1)
        in_dma(nc.sync, 2)
        in_dma(nc.scalar, 3)
        in_dma(nc.sync, 4)
        in_dma(nc.scalar, 5)
        # small, descriptor-heavy bias DMAs go on the gpsimd SWDGE queue so
        # they never clog the two critical HWDGE queues
        nc.gpsimd.dma_start(bqk[:],
                            b_d[0:2 * C].rearrange("(a p) -> p a", p=P))
        nc.gpsimd.dma_start(bv_row[:],
                            b_d[2 * C:3 * C].rearrange("(o a) -> o a", o=1))
        for k in range(KC):
            # odd k on the ACT HWDGE queue, even k on gpsimd SWDGE
            (nc.scalar if k % 2 == 1 else nc.gpsimd).dma_start(
                wv[k][:], w_d[P * k:P * (k + 1), 2 * C:3 * C])

        # ---- constants (gpsimd; PE/ACT/DVE untouched) ----
        bv = pp.tile([P, C], F32, tag="bv", name="bv")
        nc.gpsimd.partition_broadcast(bv[:], bv_row[0:1, :])

        # causal+scale mask for diagonal-start pieces:
        # cols [0,128) = (col >= part ? SCALE : 0) ; cols [128,640) = SCALE
        mask = pp.tile([P, 128 + TCH], F32, tag="mask", name="mask")
        nc.gpsimd.memset(mask[:], SCALE)
        nc.gpsimd.affine_select(
            out=mask[:, 0:P], in_=mask[:, 0:P],
            compare_op=ALU.is_ge, fill=0.0, base=0,
            pattern=[[1, P]], channel_multiplier=-1)
        # 0/1 wedge (bf16) for the two-op diag variant (ACT relu + DVE fix)
        wedge = pp.tile([P, P], BF16, tag="wedge", name="wedge")
        nc.gpsimd.memset(wedge[:], 1.0)
        nc.gpsimd.affine_select(
            out=wedge[:], in_=wedge[:],
            compare_op=ALU.is_ge, fill=0.0, base=0,
            pattern=[[1, P]], channel_multiplier=-1)

        # ---- persistent activations (bf16) ----
        qkT = [pp.tile([P, T], BF16, tag=f"qkT{m}", name=f"qkT{m}")
               for m in range(2 * NPAIR)]
        v_sb = [vp.tile([P, C], BF16, tag=f"v{i}", name=f"v{i}")
                for i in range(NT)]

        load = {"act": 0.0, "dve": 0.0}  # emission-time engine balance (ns)

        def qk_steps(m, t):
            # qkT[m][:, 512t:512(t+1)] = (W[:, 128m:128(m+1)]^T @ x^T + b)
            # as a list of single-matmul closures (chain + final ACT copy)
            # so callers can interleave the chain into score runs
            box = {}

            def step(k):
                def go():
                    if "ps" not in box:
                        box["ps"] = qps.tile([P, TCH], F32, tag="qkvps",
                                             name="qkvps")
                    nc.tensor.matmul(
                        box["ps"][:],
                        w_sb[k][:, P * m:P * (m + 1)],
                        xT[k][:, TCH * t:TCH * (t + 1)],
                        start=(k == 0), stop=(k == KC - 1))
                    if k == KC - 1:
                        nc.scalar.activation(
                            qkT[m][:, TCH * t:TCH * (t + 1)],
                            box["ps"][:], AF.Identity,
                            bias=bqk[:, m:m + 1])
                        load["act"] += 690
                return go
            return [step(k) for k in range(KC)]

        def emit_qk(m, t):
            for go in qk_steps(m, t):
                go()

        def emit_v(i):
            # v_sb[i] = x[128i:128(i+1), :] @ Wv + bv   (t-part, hd-cols)
            for (n0, n1) in ((0, TCH), (TCH, C)):
                ps = qps.tile([P, TCH], F32, tag="qkvps", name="qkvps")
                for k in range(KC):
                    nc.tensor.matmul(
                        ps[:, 0:n1 - n0],
                        xT[k][:, P * i:P * (i + 1)],
                        wv[k][:, n0:n1],
                        start=(k == 0), stop=(k == KC - 1))
                nc.vector.tensor_tensor(
                    v_sb[i][:, n0:n1], ps[:, 0:n1 - n0],
                    bv[:, n0:n1], ALU.add)

        # ======= PRE: pair-0 qk (t=0) + all of v =======
        # HAM warm-up: dummy accumulation chains on a zeroed tile keep the
        # PE "busy" window satisfied until real inputs land (cold clock is
        # 1.2 GHz until ~3.4us of sustained activity, and ~3.4us of idle
        # re-throttles) — sized to bridge the preamble->first-weights gap
        dps = a_ps.tile([P, TCH], F32, tag="aps", name="aps")
        for rep in range(4):
            for k in range(KC):
                nc.tensor.matmul(dps[:], scratch[:, 0:P], scratch[:],
                                 start=(k == 0), stop=(k == KC - 1))

        # wave-1: q/k chains for pairs 0-2, interleaved k-major so each
        # (w_k, xT_k) DMA arrival unblocks six matmuls (FIFO-friendly
        # start; borrows a_ps banks, which are idle until the windows)
        wave = [0, NPAIR, 1, NPAIR + 1, 2, NPAIR + 2]
        wps = {}
        for i, m in enumerate(wave):
            pool = qps if i < 2 else a_ps
            wps[m] = pool.tile([P, TCH], F32, tag=pool is qps and "qkvps" or "aps",
                               name="wave")
        for k in range(KC):
            for m in wave:
                nc.tensor.matmul(
                    wps[m][:],
                    w_sb[k][:, P * m:P * (m + 1)],
                    xT[k][:, 0:TCH],
                    start=(k == 0), stop=(k == KC - 1))
        for m in wave:
            nc.scalar.activation(qkT[m][:, 0:TCH], wps[m][:], AF.Identity,
                                 bias=bqk[:, m:m + 1])
            load["act"] += 690
        for i in range(NT):
            emit_v(i)

        # ======= attention, chunk-software-pipelined =======
        # window (p, c): strips r = 0..(4c+3); strip r covers q-cols
        # [max(128r, 512c), 512(c+1)) of chunk c — always starting at the
        # diagonal when 128r >= 512c.  Each window runs this chunk's score
        # matmuls interleaved 1:1 with the PREVIOUS chunk's (fully-ready)
        # AV matmuls, then this window's K=128 qkv fill block.
        def relu_piece(at, ps, n, diag):
            # pick cheapest placement for the PSUM->SBUF relu pass
            act_c = n * 0.833 + 260
            dve_c = n * 1.042 + 130
            if not diag:
                if load["act"] + act_c <= load["dve"] + dve_c:
                    load["act"] += act_c
                    nc.scalar.activation(at[:, 0:n], ps[:, 0:n],
                                         AF.Relu, scale=SCALE)
                else:
                    load["dve"] += dve_c
                    nc.vector.tensor_scalar(
                        at[:, 0:n], ps[:, 0:n], SCALE, 0.0,
                        ALU.mult, ALU.max)
                return
            # diag: one-pass masked STT on DVE, or ACT relu + DVE wedge fix
            if load["dve"] + dve_c <= load["act"] + act_c + 200:
                load["dve"] += dve_c
                nc.vector.scalar_tensor_tensor(
                    at[:, 0:n], ps[:, 0:n], 0.0,
                    mask[:, 0:n], ALU.max, ALU.mult)
            else:
                load["act"] += act_c
                load["dve"] += 200
                nc.scalar.activation(at[:, 0:n], ps[:, 0:n],
                                     AF.Relu, scale=SCALE)
                nc.vector.tensor_tensor(at[:, 0:P], at[:, 0:P],
                                        wedge[:], ALU.mult)

        def make_chunk(p, c):
            qt, kt = qkT[p], qkT[NPAIR + p]
            c_lo, c_hi = TCH * c, TCH * (c + 1)
            nstr = 4 * c + 4
            pieces = {}

            def emit_score(r):
                q0 = max(P * r, c_lo)
                n = c_hi - q0
                diag = P * r >= c_lo
                prs = []
                for hh in range(2):
                    h0 = HS * hh
                    ps = a_ps.tile([P, TCH], F32, tag="aps", name="aps")
                    nc.tensor.matmul(
                        ps[:, 0:n],
                        kt[h0:h0 + HS, P * r:P * (r + 1)],
                        qt[h0:h0 + HS, q0:c_hi],
                        start=True, stop=True,
                        tile_position=(h0, 0))
                    at = at_pool.tile([P, TCH], BF16, tag="attp",
                                      name="attp")
                    relu_piece(at, ps, n, diag)
                    prs.append((q0 - c_lo, n, at))
                pieces[r] = prs

            state = {"yp": None}

            def emit_av(r):
                if state["yp"] is None:
                    state["yp"] = [y_ps.tile([P, TCH], F32, tag="yps",
                                             name="yps") for _ in range(2)]
                yp = state["yp"]
                for hh in range(2):
                    h0 = HS * hh
                    off, n, at = pieces[r][hh]
                    nc.tensor.matmul(
                        yp[hh][h0:h0 + HS, off:off + n],
                        v_sb[r][:, P * p + h0:P * p + h0 + HS],
                        at[:, 0:n],
                        start=(r == 0), stop=(r == nstr - 1),
                        tile_position=(0, h0))

            def emit_wb(yT):
                # y^T chunk PSUM -> SBUF (cast bf16), split ACT/DVE
                yp = state["yp"]
                nc.scalar.activation(yT[0:HS, c_lo:c_hi],
                                     yp[0][0:HS, :], AF.Copy)
                nc.vector.tensor_copy(yT[HS:P, c_lo:c_hi], yp[1][HS:P, :])
                load["act"] += 687
                load["dve"] += 660

            return list(range(nstr)), emit_score, emit_av, emit_wb

        # chunk stream: (p, c) for all pairs, then a sentinel drain slot
        chunks = [(p, c) for p in range(NPAIR) for c in range(NCH)]
        yts = {}
        prev = None  # (p, c, strips, emit_av, emit_wb)
        for (p, c) in chunks:
            if c == 0:
                yts[p] = yt_pool.tile([P, T], BF16, tag="yT", name="yT")
            strips, emit_score, emit_av, emit_wb = make_chunk(p, c)

            # K=128 qkv fill chains for upcoming windows, as single-MM
            # closures interleaved into the score run (keeps PE fed while
            # the relu pass paces the score PSUM rotation)
            if c == 0:
                fills = qk_steps(p, 1) + qk_steps(NPAIR + p, 1)
            elif 3 <= p + 1 < NPAIR:  # t0 chains for pairs 0-2 in wave-1
                fills = qk_steps(p + 1, 0) + qk_steps(NPAIR + p + 1, 0)
            else:
                fills = []

            # score run: all strips, 3 qk chain-steps after each twin
            per = max(1, (len(fills) + len(strips) - 1) // len(strips))
            fi = 0
            for r in strips:
                emit_score(r)
                for _ in range(per):
                    if fi < len(fills):
                        fills[fi]()
                        fi += 1
            while fi < len(fills):
                fills[fi]()
                fi += 1

            # previous chunk's AV as one contiguous accumulation run
            # (chain steps back-to-back hide the PE drain completely)
            if prev:
                for r in prev[2]:
                    prev[3](r)
                prev[4](yts[prev[0]])
                # stream this chunk of y^T out right away
                pc_lo, pc_hi = TCH * prev[1], TCH * (prev[1] + 1)
                nc.sync.dma_start(
                    y_d[P * prev[0]:P * (prev[0] + 1), pc_lo:pc_hi],
                    yts[prev[0]][:, pc_lo:pc_hi])
            prev = (p, c, strips, emit_av, emit_wb)
        # drain last chunk
        p = prev[0]
        for r in prev[2]:
            prev[3](r)
        prev[4](yts[p])
        nc.sync.dma_start(y_d[P * p:P * (p + 1), TCH:T],
                          yts[p][:, TCH:T])


def _ensure_ntff_hook():
    """Register the axon NTFF profiling hook if the image's antenv lacks
    axon_hooks (bass_utils hard-imports it on the trace=True path)."""
    import types
    try:
        from antenv import axon_hooks  # noqa: F401
        return
    except ImportError:
        pass
    import antenv
    mod = types.ModuleType("antenv.axon_hooks")
    mod._hook = None

    def set_axon_ntff_profile_hook(h):
        mod._hook = h

    def get_axon_ntff_profile_hook():
        return mod._hook

    mod.set_axon_ntff_profile_hook = set_axon_ntff_profile_hook
    mod.get_axon_ntff_profile_hook = get_axon_ntff_profile_hook
    sys.modules["antenv.axon_hooks"] = mod
    antenv.axon_hooks = mod
    try:
        from trn_agent_boot.trn_boot import _ntff_profile_via_ctypes
        hook = _ntff_profile_via_ctypes("/opt/axon/libaxon_pjrt.so")
        if hook is not None:
            mod._hook = hook
    except Exception:
        pass


_NC_CACHE = None


def _get_nc():
    global _NC_CACHE
    if _NC_CACHE is None:
        _NC_CACHE = build_nc()
    return _NC_CACHE


def kernel(x, W_attn, b_attn, _trace=False):
    import ml_dtypes
    x = np.asarray(x).astype(ml_dtypes.bfloat16)
    xt = np.ascontiguousarray(x.transpose(0, 2, 1))  # [B, C, T]
    w = np.ascontiguousarray(np.asarray(W_attn).astype(ml_dtypes.bfloat16))
    b = np.ascontiguousarray(np.asarray(b_attn, dtype=np.float32))
    assert xt.shape == (B, C, T) and w.shape == (C, 3 * C) and b.shape == (3 * C,)

    if _trace:
        _ensure_ntff_hook()
    nc = _get_nc()
    in_maps = [{"x": xt[i], "w": w, "b": b} for i in range(B)]
    res = bass_utils.run_bass_kernel_spmd(
        nc, in_maps, core_ids=list(range(B)), trace=_trace)
    # device returns y^T [C, T]; transpose back per batch element
    y = np.stack([np.asarray(res.results[i]["y"]).astype(np.float32).T
                  for i in range(B)], axis=0)
    if _trace:
        kernel.last_result = res
    return y
